# revision 8
# baseline (speedup 1.0000x reference)
"""Trainium2 Bass kernel for AdaptivePrototypeContrastiveLoss.

Strategy
--------
Host (cheap, O(N*D) bookkeeping):
  * closed-form momentum EMA + LAPACK QR -> new prototypes  [7,256]
  * row-normalize feats, stable-sort rows by label, append 7 per-class
    sum columns (Y) so the tiny "positive" matmul rides the main sweep
  * precompute per-row constants (alpha/beta/valid/onehot)

Device (8 NeuronCores, SPMD, no collectives; all O(N^2) work):
  * row-shard: each core owns 9 row-tiles of 128 rows (72 tiles cover
    the 65 real tiles; spare tiles are zero/invalid)
  * per row-tile: G = rows @ feats^T via PE (fp32, K=256, 512-col
    chunks grouped into 1536-col PSUM supertiles)
  * ACT computes exp(A*sim + BIAS) directly from PSUM with a per-class
    segment accum_out -> per-class exp sums (class segments are
    contiguous because columns are label-sorted; identical on all
    cores so the graph stays SPMD-uniform)
  * neg_i = total - own-class (selected via shipped onehot); the
    global max subtraction is replaced by the constant M0=12.5 (the
    max only enters through ~1e-8-scale eps terms, verified offline)
  * per-core output: 128-partition partial sums of thresholded loss
Host: combine 8x[128,2] partials -> scalar.
"""

import numpy as np

import concourse.bass as bass
import concourse.tile as tile
from concourse import mybir
from concourse.bass_utils import run_bass_kernel_spmd

# ---- problem constants (hardcoded per spec) ----
TEMP = 0.08
EPS = 1e-8
GAMMA = 0.99
BETA = 0.5 * (1.0 - GAMMA)
B, D, C = 8192, 256, 7
N = B + C                      # 8199 rows/cols of the score matrix
NCORES = 8
NT = 9                         # row-tiles per core (9*8*128 = 9216 >= 8199)
ROWS_PER_CORE = NT * 128       # 1152
NPAD = 8704                    # columns padded to 17*512
NF = NPAD + 8                  # + 7 Y columns + 1 zero col
SUPER = 1536                   # psum supertile width (3 banks)
M0 = 12.5                      # constant stand-in for the global max
A_SCALE = 0.5 / float(np.float32(TEMP))
BIAS = (0.5 + EPS) / float(np.float32(TEMP)) - M0

F32 = mybir.dt.float32
ALU = mybir.AluOpType
ACTF = mybir.ActivationFunctionType


def _split_multi_waits(nc):
    """This container's walrus accepts only ONE sync wait per instruction;
    split extra waits into standalone single-wait EventSemaphore insts."""
    n_new = 0
    for func in nc.m.functions:
        for blk in func.blocks:
            new_insts = []
            for inst in blk.instructions:
                si = getattr(inst, "sync_info", None)
                waits = list(si.on_wait) if si and si.on_wait else []
                if len(waits) > 1:
                    for i, w in enumerate(waits[:-1]):
                        n_new += 1
                        ev = mybir.InstEventSemaphore(
                            name=f"{inst.name}-wsplit{i}",
                            engine=inst.engine,
                            ins=[],
                            outs=[],
                            sync_info=mybir.SyncInfo(on_wait=[w], on_update=[]),
                            bass_nofuse=True,
                        )
                        new_insts.append(ev)
                    si.on_wait = [waits[-1]]
                new_insts.append(inst)
            blk.instructions = new_insts
    return n_new


def _host_prep(features, labels, prototypes, momentums):
    features = np.asarray(features, dtype=np.float32)
    labels = np.asarray(labels).astype(np.int64)
    prototypes = np.asarray(prototypes, dtype=np.float32)
    momentums = np.asarray(momentums, dtype=np.float32)

    # ---- prototype update: closed form of the sequential EMA scan ----
    counts_feat = np.bincount(labels, minlength=C)
    rank = np.zeros(B, dtype=np.int64)
    seen = np.zeros(C, dtype=np.int64)
    for i, l in enumerate(labels):
        rank[i] = seen[l]
        seen[l] += 1
    w = BETA * (GAMMA ** (counts_feat[labels] - 1 - rank).astype(np.float64))
    S = np.zeros((C, B))
    S[labels, np.arange(B)] = w
    m_final = S @ features.astype(np.float64)
    wsum = np.bincount(labels, weights=w, minlength=C)
    m_final -= wsum[:, None] * prototypes.astype(np.float64)
    m_final += (GAMMA ** counts_feat.astype(np.float64))[:, None] * momentums.astype(
        np.float64
    )
    target = prototypes.astype(np.float64) + m_final
    q, _ = np.linalg.qr(target.T.astype(np.float32))
    new_protos = q.T.astype(np.float32)

    # ---- normalized, label-sorted gram operands ----
    feats = np.concatenate([features, new_protos], 0)
    labs = np.concatenate([labels, np.arange(C, dtype=np.int64)])
    nrm = np.linalg.norm(feats.astype(np.float64), axis=-1)
    fhat = feats.astype(np.float64) / nrm[:, None]
    perm = np.argsort(labs, kind="stable")
    fs = fhat[perm]
    ls = labs[perm]
    counts_all = np.bincount(ls, minlength=C)          # includes protos
    bounds = np.concatenate([[0], np.cumsum(counts_all)])  # class col ranges

    fs32 = fs.astype(np.float32)
    Y = np.zeros((D, 8), dtype=np.float64)
    for c in range(C):
        Y[:, c] = fs[bounds[c]:bounds[c + 1]].sum(0)

    ftpad = np.zeros((NF, D), dtype=np.float32)
    ftpad[:N] = fs32
    ftpad[NPAD:NPAD + 8] = Y.T.astype(np.float32)
    ft = np.ascontiguousarray(
        ftpad.T.reshape(2, 128, NF)
    )  # [k, partition, col] with K=256 on 2 partition tiles

    # ---- ACT sub-ranges: class segments x supertile edges (global) ----
    super_edges = list(range(0, NPAD, SUPER)) + [N]
    edges = sorted(set([int(b) for b in bounds] + super_edges))
    edges = [e for e in edges if e <= N]
    subranges = []  # (super_idx, off_in_super, length, class_id)
    for a, b in zip(edges[:-1], edges[1:]):
        if a >= N:
            break
        cls = int(np.searchsorted(bounds, a, side="right") - 1)
        sup = a // SUPER
        assert b <= min((sup + 1) * SUPER, N) or b <= (sup + 1) * SUPER
        subranges.append((sup, a - sup * SUPER, b - a, cls))
    # per-class slot ranges (contiguous in list order)
    slot_ranges = []
    for c in range(C):
        idxs = [i for i, sr in enumerate(subranges) if sr[3] == c]
        slot_ranges.append((min(idxs), max(idxs) + 1))
    n_slots = len(subranges)

    # ---- per-row constants, laid out per core ----
    cnt = counts_all[ls] - 1
    selfsim = (fs32.astype(np.float64) ** 2).sum(1)
    inv = 1.0 / (cnt.astype(np.float64) + EPS)
    alpha_all = A_SCALE * inv
    beta_all = (-A_SCALE * selfsim + BIAS * cnt) * inv

    per_core = []
    for core in range(NCORES):
        base = core * 1024
        hi = min(base + ROWS_PER_CORE, N) if core == NCORES - 1 else base + 1024
        nrows = max(0, hi - base)
        rows = np.zeros((ROWS_PER_CORE, D), dtype=np.float32)
        rows[:nrows] = fs32[base:base + nrows]
        rows_kt = np.ascontiguousarray(rows.T.reshape(2, 128, ROWS_PER_CORE))

        onehot = np.zeros((NT, 128, 8), dtype=np.float32)
        rowmeta = np.zeros((128, 27), dtype=np.float32)  # alpha|beta|valid
        for t in range(NT):
            for p in range(128):
                g = base + t * 128 + p
                if g < hi:
                    onehot[t, p, ls[g]] = 1.0
                    rowmeta[p, t] = alpha_all[g]
                    rowmeta[p, 9 + t] = beta_all[g]
                    rowmeta[p, 18 + t] = 1.0
        per_core.append(
            {"ft": ft, "rows": rows_kt, "onehot": onehot, "rowmeta": rowmeta}
        )
    return per_core, subranges, slot_ranges, n_slots


def _build_graph(subranges, slot_ranges, n_slots):
    nc = bass.Bass()
    ft_d = nc.declare_dram_parameter("ft", [2, 128, NF], F32, isOutput=False)
    rows_d = nc.declare_dram_parameter(
        "rows", [2, 128, ROWS_PER_CORE], F32, isOutput=False
    )
    oh_d = nc.declare_dram_parameter("onehot", [NT, 128, 8], F32, isOutput=False)
    meta_d = nc.declare_dram_parameter("rowmeta", [128, 27], F32, isOutput=False)
    out_d = nc.declare_dram_parameter("out", [128, 2], F32, isOutput=True)

    n_super = (NPAD + SUPER - 1) // SUPER  # 6 (last covers Y cols too)
    # column chunks per supertile: (ft_off, width, psum_off)
    super_chunks = []
    for s in range(n_super):
        lo = s * SUPER
        hi = min(lo + SUPER, NPAD)
        chunks = [(o, 512, o - lo) for o in range(lo, hi, 512)]
        if s == n_super - 1:
            chunks.append((NPAD, 8, hi - lo))  # Y columns
        super_chunks.append(chunks)
    y_psum_off = NPAD - (n_super - 1) * SUPER  # offset of Y cols in last super

    with tile.TileContext(nc) as tc:
        with (
            tc.tile_pool(name="persist", bufs=1) as persist,
            tc.tile_pool(name="ps", bufs=2, space="PSUM") as psA,
            tc.tile_pool(name="scr", bufs=2) as scrp,
            tc.tile_pool(name="slots", bufs=2) as slotp,
            tc.tile_pool(name="small", bufs=4) as small,
        ):
            # --- resident inputs ---
            rows_sb = []
            for k in range(2):
                t_ = persist.tile([128, ROWS_PER_CORE], F32, tag=f"rows{k}")
                nc.sync.dma_start(out=t_[:], in_=rows_d[k])
                rows_sb.append(t_)
            meta_sb = persist.tile([128, 27], F32, tag="meta")
            nc.sync.dma_start(out=meta_sb[:], in_=meta_d[:])
            oh_sb = persist.tile([128, NT, 8], F32, tag="oh")
            for t in range(NT):
                nc.sync.dma_start(out=oh_sb[:, t, :], in_=oh_d[t])
            ft_sb = []
            for k in range(2):
                t_ = persist.tile([128, NF], F32, tag=f"ft{k}")
                ft_sb.append(t_)
            for s in range(n_super):
                lo = s * SUPER
                hi = min(lo + SUPER, NF) if s == n_super - 1 else lo + SUPER
                if s == n_super - 1:
                    hi = NF
                for k in range(2):
                    nc.sync.dma_start(
                        out=ft_sb[k][:, lo:hi], in_=ft_d[k, :, lo:hi]
                    )

            possel9 = persist.tile([128, NT], F32, tag="possel")
            negsum9 = persist.tile([128, NT], F32, tag="negsum")
            bias_exp = persist.tile([128, 1], F32, tag="bias_exp")
            nc.vector.memset(bias_exp[:], float(BIAS))
            bias_ln = persist.tile([128, 1], F32, tag="bias_ln")
            nc.vector.memset(bias_ln[:], float(EPS))

            # --- main loop over row-tiles ---
            for t in range(NT):
                slots_t = slotp.tile([128, max(n_slots, 8)], F32, tag="slots")
                classsum = small.tile([128, 8], F32, tag="csum")
                for s in range(n_super):
                    ps = psA.tile([128, SUPER], F32, tag="ps")
                    for k in range(2):
                        for (off, w, poff) in super_chunks[s]:
                            nc.tensor.matmul(
                                ps[:, poff:poff + w],
                                lhsT=rows_sb[k][:, t * 128:(t + 1) * 128],
                                rhs=ft_sb[k][:, off:off + w],
                                start=(k == 0),
                                stop=(k == 1),
                            )
                    scr = scrp.tile([128, SUPER], F32, tag="scr")
                    for si, (sup, off, ln, cls) in enumerate(subranges):
                        if sup != s:
                            continue
                        nc.scalar.activation(
                            scr[:, off:off + ln],
                            ps[:, off:off + ln],
                            ACTF.Exp,
                            bias=bias_exp[:],
                            scale=float(A_SCALE),
                            accum_out=slots_t[:, si:si + 1],
                        )
                    if s == n_super - 1:
                        scr7 = small.tile([128, 7], F32, tag="scr7")
                        nc.vector.tensor_tensor(
                            out=scr7[:],
                            in0=ps[:, y_psum_off:y_psum_off + 7],
                            in1=oh_sb[:, t, 0:7],
                            op=ALU.mult,
                        )
                        nc.vector.reduce_sum(
                            possel9[:, t:t + 1], scr7[:], mybir.AxisListType.X
                        )
                # per-class sums -> total & own
                for c in range(C):
                    a, b = slot_ranges[c]
                    nc.vector.reduce_sum(
                        classsum[:, c:c + 1], slots_t[:, a:b],
                        mybir.AxisListType.X,
                    )
                stot = small.tile([128, 1], F32, tag="stot")
                nc.vector.reduce_sum(
                    stot[:], classsum[:, 0:7], mybir.AxisListType.X
                )
                scr7b = small.tile([128, 7], F32, tag="scr7b")
                sown = small.tile([128, 1], F32, tag="sown")
                nc.vector.tensor_tensor(
                    out=scr7b[:],
                    in0=classsum[:, 0:7],
                    in1=oh_sb[:, t, 0:7],
                    op=ALU.mult,
                )
                nc.vector.reduce_sum(sown[:], scr7b[:], mybir.AxisListType.X)
                nc.vector.tensor_tensor(
                    out=negsum9[:, t:t + 1], in0=stot[:], in1=sown[:],
                    op=ALU.subtract,
                )

            # --- epilogue: loss, threshold, partial sums ---
            alpha9 = meta_sb[:, 0:NT]
            beta9 = meta_sb[:, 9:9 + NT]
            valid9 = meta_sb[:, 18:18 + NT]
            pos9 = persist.tile([128, NT], F32, tag="pos9")
            nc.vector.tensor_tensor(
                out=pos9[:], in0=possel9[:], in1=alpha9, op=ALU.mult
            )
            nc.vector.tensor_tensor(
                out=pos9[:], in0=pos9[:], in1=beta9, op=ALU.add
            )
            neg9 = persist.tile([128, NT], F32, tag="neg9")
            nc.scalar.activation(
                neg9[:], negsum9[:], ACTF.Ln, bias=bias_ln[:], scale=1.0
            )
            loss9 = persist.tile([128, NT], F32, tag="loss9")
            nc.vector.tensor_tensor(
                out=loss9[:], in0=neg9[:], in1=pos9[:], op=ALU.subtract
            )
            gt9 = persist.tile([128, NT], F32, tag="gt9")
            nc.vector.tensor_scalar(
                out=gt9[:], in0=loss9[:], scalar1=0.0, scalar2=None,
                op0=ALU.is_gt,
            )
            nc.vector.tensor_tensor(
                out=gt9[:], in0=gt9[:], in1=valid9, op=ALU.mult
            )
            contrib9 = persist.tile([128, NT], F32, tag="contrib9")
            nc.vector.tensor_tensor(
                out=contrib9[:], in0=loss9[:], in1=gt9[:], op=ALU.mult
            )
            out_t = persist.tile([128, 2], F32, tag="out")
            nc.vector.reduce_sum(
                out_t[:, 0:1], contrib9[:], mybir.AxisListType.X
            )
            nc.vector.reduce_sum(out_t[:, 1:2], gt9[:], mybir.AxisListType.X)
            nc.sync.dma_start(out=out_d[:], in_=out_t[:])
    return nc


def _run(features, labels, prototypes, momentums, trace=False, trace_kwargs=None):
    per_core, subranges, slot_ranges, n_slots = _host_prep(
        features, labels, prototypes, momentums
    )
    nc = _build_graph(subranges, slot_ranges, n_slots)
    _split_multi_waits(nc)
    in_maps = [per_core[i] for i in range(NCORES)]
    kw = {}
    if trace:
        kw = dict(trace=True, trace_cores=list(range(NCORES)))
        if trace_kwargs:
            kw["trace_kwargs"] = trace_kwargs
    res = run_bass_kernel_spmd(nc, in_maps, core_ids=list(range(NCORES)), **kw)
    loss_sum = 0.0
    cnt_sum = 0.0
    for r in res.results:
        o = np.asarray(r["out"], dtype=np.float64)
        loss_sum += o[:, 0].sum()
        cnt_sum += o[:, 1].sum()
    val = loss_sum / max(cnt_sum, 1.0) if cnt_sum > 0 else 0.0
    return np.float32(val), res


def kernel(features, labels, prototypes, momentums):
    val, _ = _run(features, labels, prototypes, momentums)
    return np.array(val, dtype=np.float32)


# revision 10
# speedup vs baseline: 2.2835x; 2.2835x over previous
"""Trainium2 Bass kernel for AdaptivePrototypeContrastiveLoss.

Strategy
--------
Host (cheap, O(N*D) bookkeeping):
  * closed-form momentum EMA + LAPACK QR -> new prototypes  [7,256]
  * row-normalize feats, stable-sort rows by label, append 7 per-class
    sum columns (Y) so the tiny "positive" matmul rides the main sweep
  * precompute per-row constants (alpha/beta/valid/onehot)

Device (8 NeuronCores, SPMD, no collectives; all O(N^2) work):
  * row-shard: each core owns 9 row-tiles of 128 rows (72 tiles cover
    the 65 real tiles; spare tiles are zero/invalid)
  * per row-tile: G = rows @ feats^T via PE (fp32, K=256, 512-col
    chunks grouped into 1536-col PSUM supertiles)
  * ACT computes exp(A*sim + BIAS) directly from PSUM with a per-class
    segment accum_out -> per-class exp sums (class segments are
    contiguous because columns are label-sorted; identical on all
    cores so the graph stays SPMD-uniform)
  * neg_i = total - own-class (selected via shipped onehot); the
    global max subtraction is replaced by the constant M0=12.5 (the
    max only enters through ~1e-8-scale eps terms, verified offline)
  * per-core output: 128-partition partial sums of thresholded loss
Host: combine 8x[128,2] partials -> scalar.
"""

import ml_dtypes
import numpy as np

import concourse.bass as bass
import concourse.tile as tile
from concourse import mybir
from concourse.bass_utils import run_bass_kernel_spmd

# ---- problem constants (hardcoded per spec) ----
TEMP = 0.08
EPS = 1e-8
GAMMA = 0.99
BETA = 0.5 * (1.0 - GAMMA)
B, D, C = 8192, 256, 7
N = B + C                      # 8199 rows/cols of the score matrix
NCORES = 8
NT = 9                         # row-tiles per core (9*8*128 = 9216 >= 8199)
ROWS_PER_CORE = NT * 128       # 1152
NPAD = 8704                    # columns padded to 17*512
NF = NPAD + 8                  # + 7 Y columns + 1 zero col
SUPER = 1536                   # psum supertile width (3 banks)
M0 = 12.5                      # constant stand-in for the global max
A_SCALE = 0.5 / float(np.float32(TEMP))
BIAS = (0.5 + EPS) / float(np.float32(TEMP)) - M0

F32 = mybir.dt.float32
BF16 = mybir.dt.bfloat16
ALU = mybir.AluOpType
ACTF = mybir.ActivationFunctionType


def _split_multi_waits(nc):
    """This container's walrus accepts only ONE sync wait per instruction;
    split extra waits into standalone single-wait EventSemaphore insts."""
    n_new = 0
    for func in nc.m.functions:
        for blk in func.blocks:
            new_insts = []
            for inst in blk.instructions:
                si = getattr(inst, "sync_info", None)
                waits = list(si.on_wait) if si and si.on_wait else []
                if len(waits) > 1:
                    for i, w in enumerate(waits[:-1]):
                        n_new += 1
                        ev = mybir.InstEventSemaphore(
                            name=f"{inst.name}-wsplit{i}",
                            engine=inst.engine,
                            ins=[],
                            outs=[],
                            sync_info=mybir.SyncInfo(on_wait=[w], on_update=[]),
                            bass_nofuse=True,
                        )
                        new_insts.append(ev)
                    si.on_wait = [waits[-1]]
                new_insts.append(inst)
            blk.instructions = new_insts
    return n_new


def _host_prep(features, labels, prototypes, momentums):
    features = np.asarray(features, dtype=np.float32)
    labels = np.asarray(labels).astype(np.int64)
    prototypes = np.asarray(prototypes, dtype=np.float32)
    momentums = np.asarray(momentums, dtype=np.float32)

    # ---- prototype update: closed form of the sequential EMA scan ----
    counts_feat = np.bincount(labels, minlength=C)
    rank = np.zeros(B, dtype=np.int64)
    seen = np.zeros(C, dtype=np.int64)
    for i, l in enumerate(labels):
        rank[i] = seen[l]
        seen[l] += 1
    w = BETA * (GAMMA ** (counts_feat[labels] - 1 - rank).astype(np.float64))
    S = np.zeros((C, B))
    S[labels, np.arange(B)] = w
    m_final = S @ features.astype(np.float64)
    wsum = np.bincount(labels, weights=w, minlength=C)
    m_final -= wsum[:, None] * prototypes.astype(np.float64)
    m_final += (GAMMA ** counts_feat.astype(np.float64))[:, None] * momentums.astype(
        np.float64
    )
    target = prototypes.astype(np.float64) + m_final
    q, _ = np.linalg.qr(target.T.astype(np.float32))
    new_protos = q.T.astype(np.float32)

    # ---- normalized, label-sorted gram operands ----
    feats = np.concatenate([features, new_protos], 0)
    labs = np.concatenate([labels, np.arange(C, dtype=np.int64)])
    nrm = np.linalg.norm(feats.astype(np.float64), axis=-1)
    fhat = feats.astype(np.float64) / nrm[:, None]
    perm = np.argsort(labs, kind="stable")
    fs = fhat[perm]
    ls = labs[perm]
    counts_all = np.bincount(ls, minlength=C)          # includes protos
    bounds = np.concatenate([[0], np.cumsum(counts_all)])  # class col ranges

    fs32 = fs.astype(np.float32)
    Y = np.zeros((D, 8), dtype=np.float64)
    for c in range(C):
        Y[:, c] = fs[bounds[c]:bounds[c + 1]].sum(0)

    ftpad = np.zeros((NF, D), dtype=np.float32)
    ftpad[:N] = fs32
    ftpad[NPAD:NPAD + 8] = Y.T.astype(np.float32)
    ft = np.ascontiguousarray(
        ftpad.T.reshape(2, 128, NF)
    ).astype(ml_dtypes.bfloat16)  # [k, partition, col], K=256 on 2 tiles

    # ---- ACT sub-ranges: class segments x supertile edges (global) ----
    super_edges = list(range(0, NPAD, SUPER)) + [N]
    edges = sorted(set([int(b) for b in bounds] + super_edges))
    edges = [e for e in edges if e <= N]
    subranges = []  # (super_idx, off_in_super, length, class_id)
    for a, b in zip(edges[:-1], edges[1:]):
        if a >= N:
            break
        cls = int(np.searchsorted(bounds, a, side="right") - 1)
        sup = a // SUPER
        assert b <= min((sup + 1) * SUPER, N) or b <= (sup + 1) * SUPER
        subranges.append((sup, a - sup * SUPER, b - a, cls))
    # per-class slot ranges (contiguous in list order)
    slot_ranges = []
    for c in range(C):
        idxs = [i for i, sr in enumerate(subranges) if sr[3] == c]
        slot_ranges.append((min(idxs), max(idxs) + 1))
    n_slots = len(subranges)

    # ---- per-row constants, laid out per core ----
    cnt = counts_all[ls] - 1
    selfsim = (fs32.astype(np.float64) ** 2).sum(1)
    inv = 1.0 / (cnt.astype(np.float64) + EPS)
    alpha_all = A_SCALE * inv
    beta_all = (-A_SCALE * selfsim + BIAS * cnt) * inv

    per_core = []
    for core in range(NCORES):
        base = core * 1024
        hi = min(base + ROWS_PER_CORE, N) if core == NCORES - 1 else base + 1024
        nrows = max(0, hi - base)
        rows = np.zeros((ROWS_PER_CORE, D), dtype=np.float32)
        rows[:nrows] = fs32[base:base + nrows]
        rows_kt = np.ascontiguousarray(rows.T.reshape(2, 128, ROWS_PER_CORE)).astype(ml_dtypes.bfloat16)

        onehot = np.zeros((NT, 128, 8), dtype=np.float32)
        rowmeta = np.zeros((128, 27), dtype=np.float32)  # alpha|beta|valid
        for t in range(NT):
            for p in range(128):
                g = base + t * 128 + p
                if g < hi:
                    onehot[t, p, ls[g]] = 1.0
                    rowmeta[p, t] = alpha_all[g]
                    rowmeta[p, 9 + t] = beta_all[g]
                    rowmeta[p, 18 + t] = 1.0
        per_core.append(
            {"ft": ft, "rows": rows_kt, "onehot": onehot, "rowmeta": rowmeta}
        )
    return per_core, subranges, slot_ranges, n_slots


def _build_graph(subranges, slot_ranges, n_slots):
    nc = bass.Bass()
    ft_d = nc.declare_dram_parameter("ft", [2, 128, NF], BF16, isOutput=False)
    rows_d = nc.declare_dram_parameter(
        "rows", [2, 128, ROWS_PER_CORE], BF16, isOutput=False
    )
    oh_d = nc.declare_dram_parameter("onehot", [NT, 128, 8], F32, isOutput=False)
    meta_d = nc.declare_dram_parameter("rowmeta", [128, 27], F32, isOutput=False)
    out_d = nc.declare_dram_parameter("out", [128, 2], F32, isOutput=True)

    n_super = (NPAD + SUPER - 1) // SUPER  # 6 (last covers Y cols too)
    # column chunks per supertile: (ft_off, width, psum_off)
    super_chunks = []
    for s in range(n_super):
        lo = s * SUPER
        hi = min(lo + SUPER, NPAD)
        chunks = [(o, 512, o - lo) for o in range(lo, hi, 512)]
        if s == n_super - 1:
            chunks.append((NPAD, 8, hi - lo))  # Y columns
        super_chunks.append(chunks)
    y_psum_off = NPAD - (n_super - 1) * SUPER  # offset of Y cols in last super

    with tile.TileContext(nc) as tc:
        with (
            tc.tile_pool(name="persist", bufs=1) as persist,
            tc.tile_pool(name="ps", bufs=2, space="PSUM") as psA,
            tc.tile_pool(name="scr", bufs=2) as scrp,
            tc.tile_pool(name="slots", bufs=2) as slotp,
            tc.tile_pool(name="small", bufs=4) as small,
        ):
            # --- resident inputs ---
            rows_sb = []
            for k in range(2):
                t_ = persist.tile([128, ROWS_PER_CORE], BF16, tag=f"rows{k}")
                nc.sync.dma_start(out=t_[:], in_=rows_d[k])
                rows_sb.append(t_)
            meta_sb = persist.tile([128, 27], F32, tag="meta")
            nc.sync.dma_start(out=meta_sb[:], in_=meta_d[:])
            oh_sb = persist.tile([128, NT, 8], F32, tag="oh")
            for t in range(NT):
                nc.sync.dma_start(out=oh_sb[:, t, :], in_=oh_d[t])
            ft_sb = []
            for k in range(2):
                t_ = persist.tile([128, NF], BF16, tag=f"ft{k}")
                ft_sb.append(t_)
            for s in range(n_super):
                lo = s * SUPER
                hi = min(lo + SUPER, NF) if s == n_super - 1 else lo + SUPER
                if s == n_super - 1:
                    hi = NF
                for k in range(2):
                    nc.sync.dma_start(
                        out=ft_sb[k][:, lo:hi], in_=ft_d[k, :, lo:hi]
                    )

            possel9 = persist.tile([128, NT], F32, tag="possel")
            negsum9 = persist.tile([128, NT], F32, tag="negsum")
            bias_exp = persist.tile([128, 1], F32, tag="bias_exp")
            nc.vector.memset(bias_exp[:], float(BIAS))
            bias_ln = persist.tile([128, 1], F32, tag="bias_ln")
            nc.vector.memset(bias_ln[:], float(EPS))

            # --- main loop over row-tiles ---
            for t in range(NT):
                slots_t = slotp.tile([128, max(n_slots, 8)], F32, tag="slots")
                classsum = small.tile([128, 8], F32, tag="csum")
                for s in range(n_super):
                    ps = psA.tile([128, SUPER], F32, tag="ps")
                    for k in range(2):
                        for (off, w, poff) in super_chunks[s]:
                            nc.tensor.matmul(
                                ps[:, poff:poff + w],
                                lhsT=rows_sb[k][:, t * 128:(t + 1) * 128],
                                rhs=ft_sb[k][:, off:off + w],
                                start=(k == 0),
                                stop=(k == 1),
                            )
                    scr = scrp.tile([128, SUPER], F32, tag="scr")
                    sub_here = [
                        (si, sr) for si, sr in enumerate(subranges) if sr[0] == s
                    ]
                    if sub_here:
                        lo_off = min(sr[1] for _, sr in sub_here)
                        hi_off = max(sr[1] + sr[2] for _, sr in sub_here)
                        nc.scalar.activation(
                            scr[:, lo_off:hi_off],
                            ps[:, lo_off:hi_off],
                            ACTF.Exp,
                            bias=bias_exp[:],
                            scale=float(A_SCALE),
                        )
                    for si, (sup, off, ln, cls) in sub_here:
                        nc.vector.reduce_sum(
                            slots_t[:, si:si + 1], scr[:, off:off + ln],
                            mybir.AxisListType.X,
                        )
                    if s == n_super - 1:
                        scr7 = small.tile([128, 7], F32, tag="scr7")
                        nc.vector.tensor_tensor(
                            out=scr7[:],
                            in0=ps[:, y_psum_off:y_psum_off + 7],
                            in1=oh_sb[:, t, 0:7],
                            op=ALU.mult,
                        )
                        nc.vector.reduce_sum(
                            possel9[:, t:t + 1], scr7[:], mybir.AxisListType.X
                        )
                # per-class sums -> total & own
                for c in range(C):
                    a, b = slot_ranges[c]
                    nc.vector.reduce_sum(
                        classsum[:, c:c + 1], slots_t[:, a:b],
                        mybir.AxisListType.X,
                    )
                stot = small.tile([128, 1], F32, tag="stot")
                nc.vector.reduce_sum(
                    stot[:], classsum[:, 0:7], mybir.AxisListType.X
                )
                scr7b = small.tile([128, 7], F32, tag="scr7b")
                sown = small.tile([128, 1], F32, tag="sown")
                nc.vector.tensor_tensor(
                    out=scr7b[:],
                    in0=classsum[:, 0:7],
                    in1=oh_sb[:, t, 0:7],
                    op=ALU.mult,
                )
                nc.vector.reduce_sum(sown[:], scr7b[:], mybir.AxisListType.X)
                nc.vector.tensor_tensor(
                    out=negsum9[:, t:t + 1], in0=stot[:], in1=sown[:],
                    op=ALU.subtract,
                )

            # --- epilogue: loss, threshold, partial sums ---
            alpha9 = meta_sb[:, 0:NT]
            beta9 = meta_sb[:, 9:9 + NT]
            valid9 = meta_sb[:, 18:18 + NT]
            pos9 = persist.tile([128, NT], F32, tag="pos9")
            nc.vector.tensor_tensor(
                out=pos9[:], in0=possel9[:], in1=alpha9, op=ALU.mult
            )
            nc.vector.tensor_tensor(
                out=pos9[:], in0=pos9[:], in1=beta9, op=ALU.add
            )
            neg9 = persist.tile([128, NT], F32, tag="neg9")
            nc.scalar.activation(
                neg9[:], negsum9[:], ACTF.Ln, bias=bias_ln[:], scale=1.0
            )
            loss9 = persist.tile([128, NT], F32, tag="loss9")
            nc.vector.tensor_tensor(
                out=loss9[:], in0=neg9[:], in1=pos9[:], op=ALU.subtract
            )
            gt9 = persist.tile([128, NT], F32, tag="gt9")
            nc.vector.tensor_scalar(
                out=gt9[:], in0=loss9[:], scalar1=0.0, scalar2=None,
                op0=ALU.is_gt,
            )
            nc.vector.tensor_tensor(
                out=gt9[:], in0=gt9[:], in1=valid9, op=ALU.mult
            )
            contrib9 = persist.tile([128, NT], F32, tag="contrib9")
            nc.vector.tensor_tensor(
                out=contrib9[:], in0=loss9[:], in1=gt9[:], op=ALU.mult
            )
            out_t = persist.tile([128, 2], F32, tag="out")
            nc.vector.reduce_sum(
                out_t[:, 0:1], contrib9[:], mybir.AxisListType.X
            )
            nc.vector.reduce_sum(out_t[:, 1:2], gt9[:], mybir.AxisListType.X)
            nc.sync.dma_start(out=out_d[:], in_=out_t[:])
    return nc


def _run(features, labels, prototypes, momentums, trace=False, trace_kwargs=None):
    per_core, subranges, slot_ranges, n_slots = _host_prep(
        features, labels, prototypes, momentums
    )
    nc = _build_graph(subranges, slot_ranges, n_slots)
    _split_multi_waits(nc)
    in_maps = [per_core[i] for i in range(NCORES)]
    kw = {}
    if trace:
        kw = dict(trace=True, trace_cores=list(range(NCORES)))
        if trace_kwargs:
            kw["trace_kwargs"] = trace_kwargs
    res = run_bass_kernel_spmd(nc, in_maps, core_ids=list(range(NCORES)), **kw)
    loss_sum = 0.0
    cnt_sum = 0.0
    for r in res.results:
        o = np.asarray(r["out"], dtype=np.float64)
        loss_sum += o[:, 0].sum()
        cnt_sum += o[:, 1].sum()
    val = loss_sum / max(cnt_sum, 1.0) if cnt_sum > 0 else 0.0
    return np.float32(val), res


def kernel(features, labels, prototypes, momentums):
    val, _ = _run(features, labels, prototypes, momentums)
    return np.array(val, dtype=np.float32)
